# revision 40
# baseline (speedup 1.0000x reference)
"""EventVolumeSurface trilinear voxel-grid kernel for Trainium2 (Bass/Tile).

Strategy (data-parallel over batch, 1 batch -> 1 NeuronCore):
  - Host: shard events by batch id, compute bucket keys (time-segment s in
    [0,9), y-tile q in [0,4), x-tile r in [0,5)), duplicate events that
    straddle a y/x tile boundary (the trilinear hat auto-masks out-of-tile
    taps, so duplication is exact), sort into (s,q,r) buckets, pad each
    bucket to a multiple of 128 slots, and lay out slot-major [128, T]
    arrays of y, x, t, polarity.
  - Device: per event column, compute t' = a*t + b (t* in [0,9]), per
    segment frac = t' - s, kt1 = frac*pol, kt0 = pol - kt1.  Per tile of
    128 events: |IOTA_q - y| (GPSIMD), |IOTA_r - x| (DVE), hat = relu(1-d)
    (ACT, batched over groups of tiles), rhs = [kt0*hatX | kt1*hatX] (DVE),
    then one PE matmul psum[y,256] += hatY^T @ rhs accumulating the two
    adjacent bin planes of the segment.  PSUM is drained per (s,q) into an
    SBUF-resident [10,480,640] grid which is DMA'd to DRAM at the end.

The kernel program is compiled per bucket-schedule (shared across all 8
cores: per-bucket tile counts are the max over cores).
"""

import os
import sys

import numpy as np

sys.path.insert(0, "/opt/trn_rl_repo")

import concourse.bass as bass
import concourse.bacc as bacc
import concourse.mybir as mybir
import concourse.tile as tile
from concourse.bass_utils import run_bass_kernel_spmd

H, W, BINS = 480, 640, 10
NSEG = BINS - 1          # 9 time segments (events with t*=9 fold into seg 8)
P = 128
NQ = (H + P - 1) // P    # 4 y-tiles
NR = (W + P - 1) // P    # 5 x-tiles
NKEY = NSEG * NQ * NR    # 180 buckets
N_CORES = 8
GROUP = 10               # tiles per batched abs/relu/clamp op

F32 = mybir.dt.float32
F16 = mybir.dt.float16
MM_DT = F16              # PE operand dtype: fp16 is full-rate at any N
DY_GPS = bool(int(os.environ.get("EVS_DY_GPS", "1")))
TINY = bool(int(os.environ.get("EVS_TINY", "0")))  # timing diagnostic only

_prog_cache: dict = {}


def _host_prep(ev):
    """Bucket one batch's events; returns (counts[NKEY], packing arrays)."""
    if ev.shape[0] == 0:
        # degenerate batch: dummy zero-polarity events (contribute 0)
        ev = np.array([[0.0, 0.0, 0.25, 0.0, 0.0],
                       [0.0, 0.0, 0.75, 0.0, 0.0]], np.float32)
    x = ev[:, 0].astype(np.float32)
    y = ev[:, 1].astype(np.float32)
    t = ev[:, 2].astype(np.float32)
    p = ev[:, 3].astype(np.float32)
    t0 = t[0]
    tN = t[-1]
    denom = np.float32(tN - t0)
    if denom > 0:
        a = np.float32(np.float32(BINS - 1) / denom)
    else:
        a = np.float32(0.0)
    b = np.float32(-t0 * a)
    tp = (t * a + b).astype(np.float32)
    s = np.clip(np.floor(tp).astype(np.int32), 0, NSEG - 1)

    iy = np.floor(y).astype(np.int32)
    icy = np.ceil(y).astype(np.int32)
    ix = np.floor(x).astype(np.int32)
    icx = np.ceil(x).astype(np.int32)
    qf, qc = iy >> 7, icy >> 7
    rf, rc = ix >> 7, icx >> 7
    n = len(x)
    idx0 = np.arange(n, dtype=np.int64)

    ys = qf != qc
    xs = rf != rc
    both = ys & xs
    inst_idx = np.concatenate([idx0, idx0[ys], idx0[xs], idx0[both]])
    inst_q = np.concatenate([qf, qc[ys], qf[xs], qc[both]])
    inst_r = np.concatenate([rf, rf[ys], rc[xs], rc[both]])
    key = (s[inst_idx] * NQ + inst_q) * NR + inst_r
    counts = np.bincount(key, minlength=NKEY)
    return counts, (x, y, t, p, a, b, inst_idx, key)


def _pack_core(pack, tiles_per_key, T_tot):
    x, y, t, p, a, b, inst_idx, key = pack
    col0 = np.zeros(NKEY + 1, np.int64)
    col0[1:] = np.cumsum(tiles_per_key)
    order = np.argsort(key, kind="stable")
    skey = key[order]
    sidx = inst_idx[order]
    # rank within each key group
    group_start = np.searchsorted(skey, np.arange(NKEY))
    rank = np.arange(len(skey)) - group_start[skey]
    slot = col0[skey] * P + rank
    part = (slot % P).astype(np.int64)
    col = (slot // P).astype(np.int64)

    # two packed inputs: persistent y|x and prologue-only t|p|(a,b)
    YX = np.zeros((P, 2 * T_tot), np.float32)
    YX[part, col] = y[sidx]
    YX[part, T_tot + col] = x[sidx]
    TP = np.zeros((P, 2 * T_tot + 2), np.float32)
    TP[part, col] = t[sidx]
    TP[part, T_tot + col] = p[sidx]
    TP[:, 2 * T_tot] = a
    TP[:, 2 * T_tot + 1] = b
    return {"ev_yx": YX, "ev_tp": TP}


def _build_program(tiles_per_key, T_tot):
    nc = bacc.Bacc("TRN2", debug=False)
    yx_d = nc.dram_tensor("ev_yx", [P, 2 * T_tot], F32, kind="ExternalInput")
    tp_d = nc.dram_tensor("ev_tp", [P, 2 * T_tot + 2], F32,
                          kind="ExternalInput")
    out_d = nc.dram_tensor("out", [BINS, H, W], F32, kind="ExternalOutput")

    col0 = np.zeros(NKEY + 1, np.int64)
    col0[1:] = np.cumsum(tiles_per_key)
    # per-segment column ranges (keys are s-major)
    seg_c0 = [int(col0[s * NQ * NR]) for s in range(NSEG)]
    seg_c1 = [int(col0[(s + 1) * NQ * NR]) for s in range(NSEG)]

    Alu = mybir.AluOpType
    Act = mybir.ActivationFunctionType

    with tile.TileContext(nc) as tc:
        with (
            tc.tile_pool(name="persist", bufs=1) as persist,
            tc.tile_pool(name="grid", bufs=1) as gridp,
            tc.tile_pool(name="psum", bufs=2, space="PSUM") as psump,
        ):
            # --- load inputs (ev_tp only lives through the prologue)
            yxt = persist.tile([P, 2 * T_tot], F32, tag="yxt")
            yt = yxt[:, 0:T_tot]
            xt = yxt[:, T_tot:2 * T_tot]
            nc.sync.dma_start(out=yxt[:], in_=yx_d[:])

            # --- constants: per-tile iota tables 128q + c and 128r + c
            ioq = []
            ior = []
            for q in range(NQ):
                ti = persist.tile([P, P], mybir.dt.int32, tag=f"ioqi{q}")
                nc.gpsimd.iota(ti[:], pattern=[[1, P]], base=q * P,
                               channel_multiplier=0)
                tf = persist.tile([P, P], F32, tag=f"ioqf{q}")
                nc.vector.tensor_copy(tf[:], ti[:])
                ioq.append(tf)
            for r in range(NR):
                ti = persist.tile([P, P], mybir.dt.int32, tag=f"iori{r}")
                nc.gpsimd.iota(ti[:], pattern=[[1, P]], base=r * P,
                               channel_multiplier=0)
                tf = persist.tile([P, P], F32, tag=f"iorf{r}")
                nc.vector.tensor_copy(tf[:], ti[:])
                ior.append(tf)

            # --- preprocess: t' = a*t + b ; frac = t' - s ;
            #     nk1 = -frac*pol ; nk0 = -(pol - frac*pol)
            #     (negated because the muls read -hat_x: (-hat_x)*(-kt)=hat_x*kt)
            nk0 = persist.tile([P, T_tot], F32, tag="nk0")
            nk1 = persist.tile([P, T_tot], F32, tag="nk1")
            with tc.tile_pool(name="prolog", bufs=1) as prolog:
                tpt = prolog.tile([P, 2 * T_tot + 2], F32, tag="tpt")
                tt = tpt[:, 0:T_tot]
                pt = tpt[:, T_tot:2 * T_tot]
                ab = tpt[:, 2 * T_tot:2 * T_tot + 2]
                nc.sync.dma_start(out=tpt[:], in_=tp_d[:])
                tc.strict_bb_all_engine_barrier()
                nc.vector.tensor_scalar(nk1[:], tt, ab[:, 0:1], ab[:, 1:2],
                                        op0=Alu.mult, op1=Alu.add)
                for s in range(NSEG):
                    c0, c1 = seg_c0[s], seg_c1[s]
                    if c1 > c0:
                        nc.vector.tensor_scalar(nk1[:, c0:c1], nk1[:, c0:c1],
                                                float(s), None,
                                                op0=Alu.subtract)
                # nk1 holds frac; kt1 = frac*pol; nk1 := -kt1
                nc.vector.tensor_tensor(nk1[:], nk1[:], pt, op=Alu.mult)
                nc.vector.tensor_scalar(nk1[:], nk1[:], -1.0, None,
                                        op0=Alu.mult)
                # nk0 = -(pol - kt1) = -pol - nk1
                nc.vector.tensor_tensor(nk0[:], nk1[:], pt, op=Alu.add)
                nc.vector.tensor_scalar(nk0[:], nk0[:], -1.0, None,
                                        op0=Alu.mult)

            tc.strict_bb_all_engine_barrier()

            # --- the SBUF-resident output grid [128, BINS*NQ*640]
            V = gridp.tile([P, BINS * NQ * W], F32, tag="V")

            # --- main loops (EVS_REPEAT > 1 is a timing-only mode: output
            #     values are wrong for the `add` drains but timing per pass
            #     is identical)
            repeat = int(os.environ.get("EVS_REPEAT", "1"))
            with (
                tc.tile_pool(name="ay", bufs=3) as ayp,
                tc.tile_pool(name="ax", bufs=3) as axp,
                tc.tile_pool(name="hy", bufs=3) as hyp,
                tc.tile_pool(name="hx", bufs=3) as hxp,
                tc.tile_pool(name="rhs", bufs=4) as rhsp,
            ):
             for _rep in range(repeat):
              for s in range(NSEG):
                for q in range(NQ):
                    psum_t = psump.tile([P, NR * 256], F32, tag="ps")
                    for r in range(NR):
                        k = (s * NQ + q) * NR + r
                        ntile = int(tiles_per_key[k])
                        cbase = int(col0[k])
                        for g0 in range(0, ntile, GROUP):
                            gn = min(GROUP, ntile - g0)
                            gw = gn * P
                            ayg = ayp.tile([P, GROUP * P], F32, tag="ayg")
                            axg = axp.tile([P, GROUP * P], F32, tag="axg")
                            TW = 8 if TINY else P
                            for j in range(gn):
                                c = cbase + g0 + j
                                if DY_GPS:
                                    nc.gpsimd.tensor_tensor(
                                        ayg[:, j * P:j * P + TW], ioq[q][:, :TW],
                                        yt[:, c:c + 1].to_broadcast([P, TW]),
                                        op=Alu.subtract)
                                else:
                                    nc.vector.tensor_scalar(
                                        ayg[:, j * P:j * P + TW], ioq[q][:, :TW],
                                        yt[:, c:c + 1], None, op0=Alu.subtract)
                                nc.vector.tensor_scalar(
                                    axg[:, j * P:j * P + TW], ior[r][:, :TW],
                                    xt[:, c:c + 1], None, op0=Alu.subtract)
                            hyg = hyp.tile([P, GROUP * P], MM_DT, tag="hyg")
                            nhxg = hxp.tile([P, GROUP * P], MM_DT, tag="nhxg")
                            # |d| in place (ACT); hat_y = relu(1-|dy|) (ACT);
                            # -hat_x = min(|dx|-1, 0)  (DVE, batched)
                            bw = gn * P if not TINY else gn * 8
                            nc.scalar.activation(ayg[:, :bw], ayg[:, :bw],
                                                 Act.Abs)
                            nc.scalar.activation(axg[:, :bw], axg[:, :bw],
                                                 Act.Abs)
                            nc.scalar.activation(hyg[:, :bw], ayg[:, :bw],
                                                 Act.Relu, bias=1.0, scale=-1.0)
                            nc.vector.tensor_scalar(nhxg[:, :bw], axg[:, :bw],
                                                    1.0, 0.0, op0=Alu.subtract,
                                                    op1=Alu.min)
                            for j in range(gn):
                                c = cbase + g0 + j
                                rhs = rhsp.tile([P, 256], MM_DT, tag="rhs")
                                nc.vector.tensor_scalar(
                                    rhs[:, 0:TW], nhxg[:, j * P:j * P + TW],
                                    nk0[:, c:c + 1], None, op0=Alu.mult)
                                nc.vector.tensor_scalar(
                                    rhs[:, P:P + TW], nhxg[:, j * P:j * P + TW],
                                    nk1[:, c:c + 1], None, op0=Alu.mult)
                                first = (g0 + j == 0)
                                last = (g0 + j == ntile - 1)
                                nc.tensor.matmul(
                                    psum_t[:, r * 256:(r + 1) * 256],
                                    lhsT=hyg[:, j * P:(j + 1) * P],
                                    rhs=rhs[:],
                                    start=first, stop=last)
                    # drain psum -> V for plane s (half 0) and s+1 (half 1)
                    pv = psum_t[:].rearrange("p (r h c) -> p h r c", r=NR, h=2,
                                             c=P)
                    for half, plane in ((0, s), (1, s + 1)):
                        base = (plane * NQ + q) * W
                        vv = V[:, base:base + W].rearrange("p (r c) -> p r c",
                                                           c=P)
                        if (half == 0 and s == 0) or half == 1:
                            nc.scalar.copy(vv, pv[:, half])
                        else:
                            nc.vector.tensor_tensor(vv, vv, pv[:, half],
                                                    op=Alu.add)

            # --- write out
            for bin_i in range(BINS):
                for q in range(NQ):
                    rows = min(P, H - q * P)
                    base = (bin_i * NQ + q) * W
                    nc.sync.dma_start(
                        out=out_d[bin_i, q * P:q * P + rows, :],
                        in_=V[0:rows, base:base + W])
    nc.finalize()
    return nc


def kernel(events, lengths):
    events = np.ascontiguousarray(events, dtype=np.float32)
    lengths = np.asarray(lengths)
    B = int(lengths.shape[0])
    offs = np.zeros(B + 1, np.int64)
    offs[1:] = np.cumsum(lengths)

    packs = []
    counts = np.zeros((B, NKEY), np.int64)
    for bi in range(B):
        c, pk = _host_prep(events[offs[bi]:offs[bi + 1]])
        counts[bi] = c
        packs.append(pk)

    tiles_per_key = np.maximum(1, -(-counts.max(axis=0) // P)).astype(np.int64)
    T_tot = int(tiles_per_key.sum())

    key = (tuple(tiles_per_key.tolist()), T_tot,
           os.environ.get("EVS_REPEAT", "1"), TINY)
    if key not in _prog_cache:
        _prog_cache[key] = _build_program(tiles_per_key, T_tot)
    nc = _prog_cache[key]

    in_maps = [_pack_core(pk, tiles_per_key, T_tot) for pk in packs]
    trace = bool(int(os.environ.get("EVS_TRACE", "0")))
    res = run_bass_kernel_spmd(nc, in_maps, core_ids=list(range(B)),
                               trace=trace)
    global last_results
    last_results = res
    out = np.stack([r["out"] for r in res.results], axis=0)
    return out.astype(np.float32)


last_results = None


if __name__ == "__main__":
    # tiny smoke test with synthetic events
    rng = np.random.default_rng(0)
    B0, NP0 = 8, 2000
    N0 = B0 * NP0
    x = rng.uniform(0, W - 1, N0).astype(np.float32)
    y = rng.uniform(0, H - 1, N0).astype(np.float32)
    t = np.sort(rng.uniform(0, 1, (B0, NP0)).astype(np.float32), axis=1).ravel()
    p = (2.0 * rng.integers(0, 2, N0) - 1).astype(np.float32)
    b = np.repeat(np.arange(B0), NP0).astype(np.float32)
    ev = np.stack([x, y, t, p, b], axis=1)
    ln = np.full(B0, NP0, np.int32)
    out = kernel(ev, ln)
    # numpy reference
    ref = np.zeros((B0, BINS, H, W), np.float64)
    for bi in range(B0):
        sl = slice(bi * NP0, (bi + 1) * NP0)
        xx, yy, tt2, pp = x[sl], y[sl], t[sl], p[sl]
        t0, tN = tt2[0], tt2[-1]
        ts = (BINS - 1) * np.clip((tt2 - t0) / (tN - t0), 0, 1)
        import itertools
        for xr_f, yr_f, br_f in itertools.product([np.floor, np.ceil], repeat=3):
            xr, yr, br = xr_f(xx), yr_f(yy), br_f(ts)
            valid = (((xr != xx) | (xr_f is np.floor))
                     & ((yr != yy) | (yr_f is np.floor))
                     & ((br != ts) | (br_f is np.floor))
                     & (xr < W) & (yr < H) & (br < BINS))
            kb = lambda a_: np.maximum(0, 1 - np.abs(a_))
            val = np.where(valid, pp * kb(xr - xx) * kb(yr - yy) * kb(br - ts), 0)
            np.add.at(ref[bi].ravel(),
                      np.where(valid, (xr + yr * W + br * H * W).astype(np.int64), 0),
                      val)
    err = np.abs(out - ref).max() / max(1e-9, np.abs(ref).max())
    print("smoke rel err:", err)
